# revision 26
# baseline (speedup 1.0000x reference)
"""Causal self-attention (B=2, T=2048, C=2048, 16 heads) on 8 Trainium2 cores.

Sharding: tensor-parallel over heads — 2 heads per core. Each core computes
q/k/v projections for its head group, causal attention, and a partial output
projection (row-parallel Wo); the host sums the 8 partial outputs.

v5: software-pipelined emission.  The three stages are generators and a
driver interleaves their instruction emission so the (in-order) PE queue
always has independent work while the other engines catch up:

  phase 1:  qkv projections (256-token windows, 5 PSUM banks) interleaved
            with batch-0 attention windows (st/ot single-buffered, 4 banks)
  phase 2:  batch-0 out-projection (y pairs in the freed projection banks)
            interleaved with batch-1 attention
  phase 3:  batch-1 out-projection, PSUM pairs rotating over all tags

All SBUF data is bf16 (same 1 cyc/row PE rate as fp32r, half DMA/SBUF);
PSUM accumulation is fp32.  Softmax rowsums accumulate on the DVE in bf16
(one packed-mode add per k-tile covering both heads) with a final
per-window partition-sum matmul; normalization is reciprocal + broadcast +
one tensor_mul straight from the ot PSUM into bf16 ot_s.  y is written
bf16; the host sums the 8 partials in fp32.
"""

import math
import sys
from contextlib import ExitStack

import numpy as np

sys.path.insert(0, "/opt/trn_rl_repo")

import concourse.bass as bass  # noqa: E402
import concourse.tile as tile  # noqa: E402
from concourse import bacc, mybir  # noqa: E402

F32 = mybir.dt.float32
BF16 = mybir.dt.bfloat16

# Full problem constants
B_FULL, T_FULL, C_FULL = 2, 2048, 2048
N_HEADS, HEAD_DIM = 16, 128
N_CORES = 8
H_LOC = N_HEADS // N_CORES  # 2 heads per core
C_LOC = H_LOC * HEAD_DIM  # 256 output dims per core

PW = 256  # projection token window
WIN = 512  # attention q-window


def build_program(Bb=B_FULL, Tt=T_FULL, Cc=C_FULL):
    """Build the single-core program (SPMD across the 8 cores).

    Per-core DRAM interface (all bf16):
      xT : [Cc, Bb*Tt]  (x transposed, replicated)
      wq : [Cc, C_LOC]  (Wq rows for this core's heads, transposed,
                         pre-scaled by 1/sqrt(HEAD_DIM))
      wk : [Cc, C_LOC]
      wv : [Cc, C_LOC]
      wo : [C_LOC, Cc]  (Wo columns for this core's heads, transposed)
      y  : [Bb*Tt, Cc]  out (partial sum; host reduces over cores)
    """
    BT = Bb * Tt
    n_kc = Cc // 128  # contraction chunks for projections
    n_pw = BT // PW  # projection windows (16)
    n_qw = Tt // WIN  # attention q-windows per batch element
    n_bt = BT // 128  # 128-token tiles
    sub = WIN // 128  # 128-token subtiles per q-window (4)
    n_nw = Cc // WIN

    nc = bacc.Bacc("TRN2", target_bir_lowering=False, debug=False,
                   num_devices=N_CORES)

    xT_ap = nc.dram_tensor("xT", [Cc, BT], BF16, kind="ExternalInput").ap()
    wq_ap = nc.dram_tensor("wq", [Cc, C_LOC], BF16, kind="ExternalInput").ap()
    wk_ap = nc.dram_tensor("wk", [Cc, C_LOC], BF16, kind="ExternalInput").ap()
    wv_ap = nc.dram_tensor("wv", [Cc, C_LOC], BF16, kind="ExternalInput").ap()
    wo_ap = nc.dram_tensor("wo", [C_LOC, Cc], BF16, kind="ExternalInput").ap()
    y_ap = nc.dram_tensor("y", [BT, Cc], BF16, kind="ExternalOutput").ap()

    with tile.TileContext(nc) as tc, ExitStack() as ctx:
        const = ctx.enter_context(tc.tile_pool(name="const", bufs=1))
        wop = ctx.enter_context(tc.tile_pool(name="wop", bufs=1))
        qkv = ctx.enter_context(tc.tile_pool(name="qkv", bufs=1))
        wqkv = ctx.enter_context(tc.tile_pool(name="wqkv", bufs=1))
        xpool = ctx.enter_context(tc.tile_pool(name="xpool", bufs=6))
        ptpool = ctx.enter_context(tc.tile_pool(name="ptpool", bufs=3))
        accp = ctx.enter_context(tc.tile_pool(name="accp", bufs=2))
        spool = ctx.enter_context(tc.tile_pool(name="spool", bufs=2))
        ypool = ctx.enter_context(tc.tile_pool(name="ypool", bufs=8))
        ps = ctx.enter_context(tc.tile_pool(name="ps", bufs=1, space="PSUM"))

        # ones lhsT for the rowsum partition-sum matmuls
        ones_f32 = const.tile([128, 1], F32, tag="ones_f32")
        nc.any.memset(ones_f32[:], 1.0)
        ones_col = const.tile([128, 1], BF16, tag="ones_col")
        nc.vector.tensor_copy(ones_col[:], ones_f32[:])

        # Persistent SBUF tensors (bf16)
        wo_s = wop.tile([128, H_LOC, Cc], BF16, tag="wo")
        qT_s = qkv.tile([128, H_LOC, BT], BF16, tag="qT")
        kT_s = qkv.tile([128, H_LOC, BT], BF16, tag="kT")
        v_s = qkv.tile([128, n_bt, C_LOC], BF16, tag="v")
        ot_s = qkv.tile([128, H_LOC, BT], BF16, tag="ot_s")

        wq_s = wqkv.tile([128, n_kc, C_LOC], BF16, tag="wq")
        wk_s = wqkv.tile([128, n_kc, C_LOC], BF16, tag="wk")
        wv_s = wqkv.tile([128, n_kc, C_LOC], BF16, tag="wv")

        def dma_weights(kc):
            ksl = slice(kc * 128, (kc + 1) * 128)
            nc.sync.dma_start(wq_s[:, kc, :], wq_ap[ksl, :])
            nc.sync.dma_start(wk_s[:, kc, :], wk_ap[ksl, :])
            nc.sync.dma_start(wv_s[:, kc, :], wv_ap[ksl, :])

        # ---- stage 1: q/k/v projections over 256-token windows ---------
        def gen_stage1():
            for w in range(n_pw):
                toks = slice(w * PW, (w + 1) * PW)
                qk = ps.tile([128, 4, PW], F32, tag="qk", bufs=1,
                             name="qk_ps")
                vps = ps.tile([128, 2, PW], F32, tag="v", bufs=2,
                              name="v_ps")
                for kc in range(n_kc):
                    if w == 0:
                        dma_weights(kc)
                    strip = xpool.tile([128, PW], BF16, tag="strip")
                    nc.sync.dma_start(strip[:],
                                      xT_ap[kc * 128:(kc + 1) * 128, toks])
                    # PSUM `start` clears has_written for the WHOLE bank, so
                    # only the bank's first-touch matmul may carry it: the
                    # second plane sharing the bank starts with cleared bits
                    # and overwrites via the per-element has_written rule.
                    sp = (kc == n_kc - 1)
                    for h in range(H_LOC):
                        st_ = (kc == 0) and h == 0
                        hs = slice(h * 128, (h + 1) * 128)
                        nc.tensor.matmul(qk[:, h, :], wq_s[:, kc, hs],
                                         strip[:], start=st_, stop=sp,
                                         skip_group_check=True)
                        nc.tensor.matmul(qk[:, 2 + h, :], wk_s[:, kc, hs],
                                         strip[:], start=st_, stop=sp,
                                         skip_group_check=True)
                    for j in range(2):
                        nc.tensor.matmul(vps[:, j, :],
                                         strip[:, j * 128:(j + 1) * 128],
                                         wv_s[:, kc, :],
                                         start=(kc == 0) and j == 0, stop=sp,
                                         skip_group_check=True)
                    yield ("chunk", w)
                for h in range(H_LOC):
                    nc.scalar.copy(qT_s[:, h, toks], qk[:, h, :])
                    nc.scalar.copy(kT_s[:, h, toks], qk[:, 2 + h, :])
                for j in range(2):
                    nc.vector.tensor_copy(v_s[:, 2 * w + j, :], vps[:, j, :])
                yield ("w_done", w)

        # ---- stage 2: attention for one batch element -------------------
        def gen_attention(b):
            for qw in range(n_qw):
                qoff = b * Tt + qw * WIN
                qsl = slice(qoff, qoff + WIN)
                n_kt = sub * (qw + 1)
                # stage1 windows covering this batch's tokens up to the
                # causal frontier must be emitted first
                yield ("need_w", b * (Tt // PW) + 2 * qw + 1)
                ot_ps = ps.tile([128, H_LOC, WIN], F32, tag="ot", bufs=1,
                                name="ot_ps")
                acc = accp.tile([128, H_LOC, WIN], BF16, tag="acc",
                                name="acc")
                for kt in range(n_kt):
                    vs = max(0, (kt - qw * sub) * 128)
                    koff = b * Tt + kt * 128
                    stt = ps.tile([128, H_LOC, WIN], F32, tag="st", bufs=1,
                                  name="st_ps")
                    for h in range(H_LOC):
                        nc.tensor.matmul(
                            stt[:, h, vs:], kT_s[:, h, koff:koff + 128],
                            qT_s[:, h, qoff + vs:qoff + WIN],
                            start=True, stop=True)
                    pt = ptpool.tile([128, H_LOC, WIN], BF16, tag="pt",
                                     name="pt")
                    nc.scalar.activation(pt[:, :, vs:], stt[:, :, vs:],
                                         mybir.ActivationFunctionType.Exp)
                    if kt >= qw * sub:
                        # causal mask on the 128-wide diagonal band
                        for h in range(H_LOC):
                            nc.gpsimd.affine_select(
                                out=pt[:, h, vs:vs + 128],
                                in_=pt[:, h, vs:vs + 128],
                                compare_op=mybir.AluOpType.is_ge,
                                fill=0.0, base=0,
                                pattern=[[1, 128]],
                                channel_multiplier=-1,
                            )
                    first, last = (kt == 0), (kt == n_kt - 1)
                    vt = b * (Tt // 128) + kt
                    for h in range(H_LOC):
                        nc.tensor.matmul(ot_ps[:, h, vs:],
                                         v_s[:, vt, h * 128:(h + 1) * 128],
                                         pt[:, h, vs:],
                                         start=first, stop=last)
                    if first:
                        nc.vector.tensor_copy(acc[:], pt[:])
                    else:
                        nc.vector.tensor_add(acc[:, :, vs:], acc[:, :, vs:],
                                             pt[:, :, vs:])
                    yield ("kt", kt)
                # rowsum partition-sums into a spare st-tag tile, then
                # normalize straight out of the ot PSUM
                rs_mm = ps.tile([128, H_LOC, WIN], F32, tag="st", bufs=1,
                                name="rs_mm")
                for h in range(H_LOC):
                    nc.tensor.matmul(rs_mm[0:1, h, :], ones_col[:],
                                     acc[:, h, :], start=True, stop=True)
                for h in range(H_LOC):
                    srec = spool.tile([1, WIN], F32, tag="srec", name="srec")
                    nc.vector.reciprocal_approx_fast(srec[:],
                                                     rs_mm[0:1, h, :])
                    bc_sb = spool.tile([128, WIN], F32, tag="bc",
                                       name="bc_sb")
                    nc.gpsimd.partition_broadcast(bc_sb[:], srec[:])
                    nc.vector.tensor_mul(ot_s[:, h, qsl], ot_ps[:, h, :],
                                         bc_sb[:])
                yield ("norm", qw)

        # ---- stage 3: out-projection for one batch ----------------------
        def gen_outproj(b, tags):
            ti = 0
            for i, bt in enumerate(range(b * (Tt // 128),
                                         (b + 1) * (Tt // 128))):
                rows = slice(bt * 128, (bt + 1) * 128)
                for pw2 in range(n_nw // 2):
                    tag = tags[ti % len(tags)]
                    ti += 1
                    if tag == "qk":
                        yp = ps.tile([128, 4, PW], F32, tag="qk", bufs=1,
                                     name="y_ps")
                        planes = [yp[:, 0:2, :], yp[:, 2:4, :]]
                        y_sb = ypool.tile([128, 4, PW], BF16, tag="ysb_qk")
                    else:
                        yp = ps.tile([128, H_LOC, WIN], F32, tag=tag,
                                     bufs=1, name="y_ps")
                        planes = [yp[:, 0, :], yp[:, 1, :]]
                        y_sb = ypool.tile([128, H_LOC, WIN], BF16,
                                          tag="ysb")
                    for j in range(2):
                        cols = slice((2 * pw2 + j) * WIN,
                                     (2 * pw2 + j + 1) * WIN)
                        for hc in range(H_LOC):
                            nc.tensor.matmul(planes[j], ot_s[:, hc, rows],
                                             wo_s[:, hc, cols],
                                             start=(hc == 0),
                                             stop=(hc == H_LOC - 1))
                    # paired eviction: one op covers both banks
                    if ti % 2 == 0:
                        nc.vector.tensor_copy(y_sb[:], yp[:])
                    else:
                        nc.scalar.copy(y_sb[:], yp[:])
                    nc.sync.dma_start(
                        y_ap[rows, 2 * pw2 * WIN:(2 * pw2 + 2) * WIN],
                        y_sb[:])
                    yield ("pair", bt)

        # ---- driver: interleave emission --------------------------------
        s1 = gen_stage1()
        state = {"w": -1, "live": True}

        def pump_s1(n):
            for _ in range(n):
                if not state["live"]:
                    return
                ev = next(s1, None)
                if ev is None:
                    state["live"] = False
                elif ev[0] == "w_done":
                    state["w"] = ev[1]

        def pump_s1_until(w):
            while state["live"] and state["w"] < w:
                pump_s1(1)

        SEQUENTIAL = False
        if SEQUENTIAL:
            pump_s1(10 ** 6)
            for hc in range(H_LOC):
                nc.sync.dma_start(wo_s[:, hc, :],
                                  wo_ap[hc * 128:(hc + 1) * 128, :])
            for _ in gen_attention(0):
                pass
            for _ in gen_outproj(0, ["qk"]):
                pass
            for _ in gen_attention(1):
                pass
            for _ in gen_outproj(1, ["qk", "st", "ot"]):
                pass
        else:
            # phase 1: attention b0, stage1 spread through it
            for ev in gen_attention(0):
                if ev[0] == "need_w":
                    pump_s1_until(ev[1])
                else:
                    pump_s1(6)
            # drain remaining stage1 windows; wo arrives during the drain
            for hc in range(H_LOC):
                nc.sync.dma_start(wo_s[:, hc, :],
                                  wo_ap[hc * 128:(hc + 1) * 128, :])
            pump_s1(10 ** 6)

            # phase 2: attention b1 with batch-0 out-projection as filler
            o0 = gen_outproj(0, ["qk"])
            for ev in gen_attention(1):
                if ev[0] == "kt":
                    next(o0, None)
            for _ in o0:
                pass

            # phase 3: batch-1 out-projection over all PSUM tags
            for _ in gen_outproj(1, ["qk", "st", "ot"]):
                pass

    nc.compile()
    return nc


_PROGRAM = None


def _get_program():
    global _PROGRAM
    if _PROGRAM is None:
        _PROGRAM = build_program()
    return _PROGRAM


def make_in_maps(x, Wq, Wk, Wv, Wo):
    """Host-side sharding: build the per-core input dicts (bf16)."""
    import ml_dtypes
    bf16 = ml_dtypes.bfloat16
    x = np.asarray(x, dtype=np.float32)
    Wq = np.asarray(Wq, dtype=np.float32)
    Wk = np.asarray(Wk, dtype=np.float32)
    Wv = np.asarray(Wv, dtype=np.float32)
    Wo = np.asarray(Wo, dtype=np.float32)
    BT = x.shape[0] * x.shape[1]
    xT = np.ascontiguousarray(x.reshape(BT, -1).T).astype(bf16)
    scale = 1.0 / math.sqrt(HEAD_DIM)
    in_maps = []
    for c in range(N_CORES):
        rows = slice(c * C_LOC, (c + 1) * C_LOC)
        in_maps.append({
            "xT": xT,
            "wq": (np.ascontiguousarray(Wq[rows, :].T) * scale).astype(bf16),
            "wk": np.ascontiguousarray(Wk[rows, :].T).astype(bf16),
            "wv": np.ascontiguousarray(Wv[rows, :].T).astype(bf16),
            "wo": np.ascontiguousarray(Wo[:, rows].T).astype(bf16),
        })
    return in_maps


def reduce_out(res, inputs):
    x = np.asarray(inputs["x"])
    Bb, Tt, Cc = x.shape
    y = np.zeros((Bb * Tt, Cc), dtype=np.float32)
    for c in range(N_CORES):
        y += res.results[c]["y"].astype(np.float32)
    return y.reshape(Bb, Tt, Cc)


def kernel(x, Wq, Wk, Wv, Wo):
    from concourse.bass_utils import run_bass_kernel_spmd

    nc = _get_program()
    in_maps = make_in_maps(x, Wq, Wk, Wv, Wo)
    res = run_bass_kernel_spmd(nc, in_maps, list(range(N_CORES)))
    return reduce_out(res, {"x": x})
